# revision 1
# baseline (speedup 1.0000x reference)
"""Grayscale + single-level 2x2 Haar DWT kernel for Trainium2 (8 cores, SPMD).

Full input x [16,3,1024,1024] f32 -> out [16,4,512,512] f32.
Batch-sharded: core i handles samples [2i, 2i+1].

Math per sample (BGR weights w=(0.114,0.587,0.299), all bands scaled by 0.5):
  gray = w0*x[0] + w1*x[1] + w2*x[2]
  a,b,c,d = gray[0::2,0::2], gray[0::2,1::2], gray[1::2,0::2], gray[1::2,1::2]
  cA,cH,cV,cD = 0.5*(a+b+c+d), 0.5*(a+b-c-d), 0.5*(a-b+c-d), 0.5*(a-b-c+d)

Per band of 128 output rows (= 256 input rows), all ops in-place where legal:
  E_ch/O_ch = even/odd input rows [128,1024] via strided DMA
  E0 += r1*E1 ; E0 += r2*E2  (r_i = w_i/w_0)    - scalar_tensor_tensor on DVE
  E0 *= w0/2   (ACT engine)                      - same for O0
  drow = E0 - O0 ; E0 += O0 (=srow)              - tensor_tensor on DVE
  cA/cV = E0[:,0::2] +/- E0[:,1::2] ; cH/cD = drow[:,0::2] +/- drow[:,1::2]
"""

import numpy as np

N_CORES = 8
B, C, H, W = 16, 3, 1024, 1024
HO, WO = H // 2, W // 2
SPC = B // N_CORES  # samples per core

W_BGR = (0.114, 0.587, 0.299)

_compiled = None


def _build():
    from concourse import bacc, mybir
    from concourse.tile import TileContext

    f32 = mybir.dt.float32
    add = mybir.AluOpType.add
    sub = mybir.AluOpType.subtract
    mult = mybir.AluOpType.mult

    w0, w1, w2 = W_BGR
    r1 = w1 / w0
    r2 = w2 / w0
    w0h = w0 * 0.5

    nc = bacc.Bacc("TRN2", target_bir_lowering=False, debug=False)
    x = nc.declare_dram_parameter("x", [SPC, C, H, W], f32, isOutput=False)
    out = nc.declare_dram_parameter("out", [SPC, 4, HO, WO], f32, isOutput=True)

    n_bands = H // 256  # bands of 128 output rows per sample

    with TileContext(nc) as tc:
        with (
            tc.tile_pool(name="in_pool", bufs=4) as in_pool,
            tc.tile_pool(name="mid_pool", bufs=4) as mid_pool,
            tc.tile_pool(name="out_pool", bufs=5) as out_pool,
        ):
            for s in range(SPC):
                for b in range(n_bands):
                    r0 = b * 256
                    acc = []  # accumulated (unscaled) gray tile per parity
                    for par in range(2):  # 0: even rows, 1: odd rows
                        ch_tiles = []
                        for ch in range(C):
                            t = in_pool.tile([128, W], f32, tag=f"in{par}{ch}")
                            nc.sync.dma_start(
                                out=t[:, :], in_=x[s, ch, r0 + par : r0 + 256 : 2, :]
                            )
                            ch_tiles.append(t)
                        # separate accumulator so input tiles release right
                        # after their stt read (keeps the input DMA slots
                        # recycling fast; in-place on ch0 holds its slot for
                        # the whole band and stalls loads 4 bands ahead)
                        g = mid_pool.tile([128, W], f32, tag=f"g{par}")
                        nc.vector.scalar_tensor_tensor(
                            g[:, :], ch_tiles[1][:, :], r1, ch_tiles[0][:, :], mult, add
                        )
                        nc.vector.scalar_tensor_tensor(
                            g[:, :], ch_tiles[2][:, :], r2, g[:, :], mult, add
                        )
                        acc.append(g)
                    gE, gO = acc
                    drow = mid_pool.tile([128, W], f32, tag="drow")
                    nc.vector.tensor_tensor(drow[:, :], gE[:, :], gO[:, :], sub)
                    # srow overwrites gE (WAR on drow's read handled by Tile)
                    nc.vector.tensor_tensor(gE[:, :], gE[:, :], gO[:, :], add)
                    srow = gE

                    for sub_i, (src, op) in enumerate(
                        ((srow, add), (drow, add), (srow, sub), (drow, sub))
                    ):
                        # order: cA(srow,+), cH(drow,+), cV(srow,-), cD(drow,-)
                        o = out_pool.tile([128, WO], f32, tag=f"o{sub_i}")
                        nc.vector.tensor_tensor(
                            o[:, :], src[:, 0:W:2], src[:, 1:W:2], op
                        )
                        # w0/2 scale applied in-place on ACT (downstream of
                        # DVE); store issued from the ACT HWDGE ring so loads
                        # (SP ring) and stores use separate FIFOs.
                        nc.scalar.mul(o[:, :], o[:, :], w0h)
                        nc.scalar.dma_start(
                            out=out[s, sub_i, b * 128 : b * 128 + 128, :], in_=o[:, :]
                        )
    nc.finalize()
    return nc


def kernel(x: np.ndarray) -> np.ndarray:
    global _compiled
    from concourse.bass_utils import run_bass_kernel_spmd

    if _compiled is None:
        _compiled = _build()
    nc = _compiled

    x = np.ascontiguousarray(x, dtype=np.float32)
    in_maps = [
        {"x": x[i * SPC : (i + 1) * SPC]} for i in range(N_CORES)
    ]
    res = run_bass_kernel_spmd(nc, in_maps, list(range(N_CORES))).results
    out = np.concatenate([r["out"] for r in res], axis=0)
    return out



# revision 3
# speedup vs baseline: 1.0825x; 1.0825x over previous
"""Grayscale + single-level 2x2 Haar DWT kernel for Trainium2 (8 cores, SPMD).

Full input x [16,3,1024,1024] f32 -> out [16,4,512,512] f32.
Batch-sharded: core i handles samples [2i, 2i+1].

Math per sample (BGR weights w=(0.114,0.587,0.299), all bands scaled by 0.5):
  gray = w0*x[0] + w1*x[1] + w2*x[2]
  a,b,c,d = gray[0::2,0::2], gray[0::2,1::2], gray[1::2,0::2], gray[1::2,1::2]
  cA,cH,cV,cD = 0.5*(a+b+c+d), 0.5*(a+b-c-d), 0.5*(a-b+c-d), 0.5*(a-b-c+d)

Layout: a "superband" is 512 consecutive input rows loaded as one 2MB
contiguous DMA into a [128, 4, 1024] tile (partition p holds rows
4p..4p+3). Two superbands cover one sample plane.

Engine split (per superband):
  ACT : c_i = x_i * (w_i/2) cast f32->fp16 (scale folded into the cast;
        3 ACTIVATE-Copy ops) - keeps the 1x-only STT off the DVE.
  DVE : t = c0+c1 ; g = t+c2 (fp16 dense TT, 2x_1p mode)
        srow/drow = g[:,0::2,:] +/- g[:,1::2,:] (fp16 dense, 2x)
        cA,cH,cV,cD = srow/drow[...,0::2] +/- [...,1::2] (f32 out, 1x)
  SYNC: all DMA descriptor gen (2MB loads, 512KB stores) on the SP HWDGE
        ring; software-pipelined so loads for superband i+1 are issued
        before compute of superband i.
"""

import numpy as np

N_CORES = 8
B, C, H, W = 16, 3, 1024, 1024
HO, WO = H // 2, W // 2
SPC = B // N_CORES  # samples per core
SB = 2              # superbands per sample plane (512 input rows each)

W_BGR = (0.114, 0.587, 0.299)

_compiled = None


def _build():
    from concourse import bacc, mybir
    from concourse.tile import TileContext

    f32 = mybir.dt.float32
    f16 = mybir.dt.float16
    add = mybir.AluOpType.add
    sub = mybir.AluOpType.subtract

    nc = bacc.Bacc("TRN2", target_bir_lowering=False, debug=False)
    # same bytes as [SPC, C, H, W] f32, pre-shaped for superband DMA
    x = nc.declare_dram_parameter("x", [SPC, C, SB, 128, 4, W], f32, isOutput=False)
    out = nc.declare_dram_parameter(
        "out", [SPC, 4, SB, 128, 2, WO], f32, isOutput=True
    )

    chunks = [(s, sb) for s in range(SPC) for sb in range(SB)]

    with TileContext(nc) as tc:
        with (
            tc.tile_pool(name="in_pool", bufs=2) as in_pool,
            tc.tile_pool(name="sc_pool", bufs=2) as sc_pool,
            tc.tile_pool(name="mid_pool", bufs=2) as mid_pool,
            tc.tile_pool(name="out_pool", bufs=1) as out_pool,
        ):
            in_tiles = {}

            def issue_loads(i):
                s, sb = chunks[i]
                ts = []
                for ch in range(C):
                    t = in_pool.tile([128, 4, W], f32, tag=f"in{ch}")
                    nc.sync.dma_start(out=t[:, :, :], in_=x[s, ch, sb])
                    ts.append(t)
                in_tiles[i] = ts

            def compute_and_store(i):
                s, sb = chunks[i]
                ch_t = in_tiles.pop(i)
                sc = []
                for ch in range(C):
                    c = sc_pool.tile([128, 4, W], f16, tag=f"sc{ch}")
                    nc.scalar.mul(c[:, :, :], ch_t[ch][:, :, :], W_BGR[ch] * 0.5)
                    sc.append(c)
                t = mid_pool.tile([128, 4, W], f16, tag="t")
                nc.vector.tensor_tensor(t[:, :, :], sc[0][:, :, :], sc[1][:, :, :], add)
                # g reuses sc[0]'s slot (dead after the first TT)
                g = sc[0]
                nc.vector.tensor_tensor(g[:, :, :], t[:, :, :], sc[2][:, :, :], add)
                srow = mid_pool.tile([128, 2, W], f16, tag="srow")
                drow = mid_pool.tile([128, 2, W], f16, tag="drow")
                nc.vector.tensor_tensor(
                    srow[:, :, :], g[:, 0:4:2, :], g[:, 1:4:2, :], add
                )
                nc.vector.tensor_tensor(
                    drow[:, :, :], g[:, 0:4:2, :], g[:, 1:4:2, :], sub
                )
                for band, (src, op) in enumerate(
                    ((srow, add), (drow, add), (srow, sub), (drow, sub))
                ):
                    # band order: cA, cH, cV, cD
                    o = out_pool.tile([128, 2, WO], f32, tag=f"o{band}")
                    nc.vector.tensor_tensor(
                        o[:, :, :], src[:, :, 0:W:2], src[:, :, 1:W:2], op
                    )
                    nc.sync.dma_start(out=out[s, band, sb], in_=o[:, :, :])

            for i in range(len(chunks)):
                issue_loads(i)
                if i >= 1:
                    compute_and_store(i - 1)
            compute_and_store(len(chunks) - 1)
    nc.finalize()
    return nc


def kernel(x: np.ndarray) -> np.ndarray:
    global _compiled
    from concourse.bass_utils import run_bass_kernel_spmd

    if _compiled is None:
        _compiled = _build()
    nc = _compiled

    x = np.ascontiguousarray(x, dtype=np.float32)
    in_maps = [{"x": x[i * SPC : (i + 1) * SPC]} for i in range(N_CORES)]
    res = run_bass_kernel_spmd(nc, in_maps, list(range(N_CORES))).results
    out = np.concatenate(
        [r["out"].reshape(SPC, 4, HO, WO) for r in res], axis=0
    )
    return out
